# revision 7
# baseline (speedup 1.0000x reference)
"""Trainium2 Bass kernel for DescriptorMatcher (nn): row-sharded cdist + row min/argmin.

Strategy (8 NeuronCores, desc1 rows sharded 2048/core, desc2 replicated):
  - Distance identity: argmin_j d2(i,j) = argmax_j s(i,j),  s = 2*a.b_j - ||b_j||^2,
    d2 = ||a_i||^2 - max_j s.
  - s is produced tile-by-tile in PSUM by the TensorEngine: desc1 is pre-scaled
    by 2 and split into fp16 hi/lo halves (three matmul terms hi*hi, hi*lo,
    lo*hi -> ~1e-5 absolute accuracy at full bf16-rate PE throughput), plus a
    K=2 augmentation matmul that subtracts ||b||^2 (fp16 hi/lo rows times -1).
  - ScalarEngine evacuates PSUM->SBUF; VectorEngine does per-tile row-max
    (tensor_reduce), then max8 + max_index recover the row argmax index.
  - Outputs per core: match_dists slice [2048,1] fp32, argmin slice [2048,1] i32.
"""

import sys

import numpy as np

if "/opt/trn_rl_repo" not in sys.path:
    sys.path.append("/opt/trn_rl_repo")

B1, B2, D = 16384, 16384, 256
N_CORES = 8
P = 128          # partitions
NT = 512         # psum tile free dim (one bank)
N_HALF = 2       # process desc2 columns in halves to fit SBUF


def build_nc(B1c=B1 // N_CORES, B2_=B2, D_=D):
    import concourse.bass as bass
    import concourse.mybir as mybir
    import concourse.tile as tile

    dt = mybir.dt
    Alu = mybir.AluOpType
    Act = mybir.ActivationFunctionType
    AxX = mybir.AxisListType.X

    KC = D_ // P                 # contraction chunks (2)
    HW = B2_ // N_HALF           # half width (8192)
    NTH = HW // NT               # psum tiles per half (16)
    MB = B1c // P                # row blocks per core (16)
    GRP = min(4, NTH)            # psum tiles per matmul group
    assert NTH % GRP == 0 and NTH >= 8

    nc = bass.Bass("TRN2", target_bir_lowering=False, debug=False)

    d1t_h = nc.dram_tensor("d1t_h", [D_, B1c], dt.float16, kind="ExternalInput").ap()
    d1t_l = nc.dram_tensor("d1t_l", [D_, B1c], dt.float16, kind="ExternalInput").ap()
    d2t_h = nc.dram_tensor("d2t_h", [D_, B2_], dt.float16, kind="ExternalInput").ap()
    d2t_l = nc.dram_tensor("d2t_l", [D_, B2_], dt.float16, kind="ExternalInput").ap()
    b2a = nc.dram_tensor("b2a", [2, B2_], dt.float16, kind="ExternalInput").ap()
    a2 = nc.dram_tensor("a2", [B1c, 1], dt.float32, kind="ExternalInput").ap()
    out_d = nc.dram_tensor("out_d", [B1c, 1], dt.float32, kind="ExternalOutput").ap()
    out_i = nc.dram_tensor("out_i", [B1c, 1], dt.int32, kind="ExternalOutput").ap()

    with tile.TileContext(nc) as tc:
        with (
            tc.tile_pool(name="const", bufs=1) as cpool,
            tc.tile_pool(name="rhs", bufs=1) as rpool,
            tc.tile_pool(name="sprime", bufs=1) as spool,
            tc.tile_pool(name="mmax", bufs=2) as mpool,
            tc.tile_pool(name="small", bufs=8) as smallp,
            tc.tile_pool(name="outs", bufs=4) as opool,
            tc.tile_pool(name="psum", bufs=8, space="PSUM") as ppool,
        ):
            # --- persistent SBUF: lhsT (fp16 hi/lo per K-chunk), a2, aug consts
            lhs = {}
            for part, src in (("h", d1t_h), ("l", d1t_l)):
                for k in range(KC):
                    t = cpool.tile([P, B1c], dt.float16, tag=f"lhs_{part}{k}",
                                   name=f"lhs_{part}{k}")
                    nc.sync.dma_start(out=t[:], in_=src[k * P:(k + 1) * P, :])
                    lhs[(part, k)] = t
            a2sb = cpool.tile([P, MB], dt.float32, tag="a2sb", name="a2sb")
            nc.sync.dma_start(
                out=a2sb[:], in_=a2.rearrange("(m p) o -> p (m o)", p=P)
            )
            negones = cpool.tile([P, P], dt.float16, tag="negones", name="negones")
            nc.vector.memset(negones[:], -1.0)
            g0 = cpool.tile([P, MB], dt.float32, tag="g0", name="g0")
            i0 = cpool.tile([P, MB], dt.float32, tag="i0", name="i0")

            for h in range(N_HALF):
                cs = h * HW  # column start
                rhs = {}
                for part, src in (("h", d2t_h), ("l", d2t_l)):
                    for k in range(KC):
                        t = rpool.tile([P, HW], dt.float16, tag=f"rhs_{part}{k}",
                                       name=f"rhs_{part}{k}")
                        nc.sync.dma_start(
                            out=t[:], in_=src[k * P:(k + 1) * P, cs:cs + HW]
                        )
                        rhs[(part, k)] = t
                aug = rpool.tile([P, HW], dt.float16, tag="aug", name="aug")
                nc.sync.dma_start(out=aug[0:2, :], in_=b2a[:, cs:cs + HW])

                for m in range(MB):
                    ms = m * P
                    sp = spool.tile([P, HW], dt.float32, tag="sp", name="sp")
                    mt = mpool.tile([P, NTH], dt.float32, tag="mt", name="mt")
                    psums = [None] * NTH
                    for g in range(NTH // GRP):
                        tiles = list(range(g * GRP, (g + 1) * GRP))
                        for n in tiles:
                            psums[n] = ppool.tile(
                                [P, NT], dt.float32, tag="ps", name="ps"
                            )
                        # aug matmul: psum = -(b2h + b2l) per column
                        for n in tiles:
                            nc.tensor.matmul(
                                psums[n][:],
                                negones[0:2, :],
                                aug[0:2, n * NT:(n + 1) * NT],
                                start=True, stop=False,
                            )
                        # weight-major data matmuls: lhsT loaded once per group
                        steps = [
                            ("h", 0, ("h", "l")),
                            ("h", 1, ("h", "l")),
                            ("l", 0, ("h",)),
                            ("l", 1, ("h",)),
                        ]
                        for si, (wp, wk, rparts) in enumerate(steps):
                            last_step = si == len(steps) - 1
                            w = lhs[(wp, wk)][:, ms:ms + P]
                            for ri, rp in enumerate(rparts):
                                last = last_step and ri == len(rparts) - 1
                                for n in tiles:
                                    r = rhs[(rp, wk)][:, n * NT:(n + 1) * NT]
                                    nc.tensor.matmul(
                                        psums[n][:], w, r,
                                        start=False, stop=last,
                                    )
                        for n in tiles:
                            nc.scalar.copy(sp[:, n * NT:(n + 1) * NT], psums[n][:])
                            nc.vector.tensor_reduce(
                                mt[:, n:n + 1], psums[n][:], AxX, Alu.max
                            )
                    mx8 = smallp.tile([P, 8], dt.float32, tag="mx8", name="mx8")
                    ix8 = smallp.tile([P, 8], dt.uint32, tag="ix8", name="ix8")
                    nc.vector.max(mx8[:], mt[:])
                    nc.vector.max_index(ix8[:], mx8[:], sp[:])
                    if h == 0:
                        nc.vector.tensor_copy(g0[:, m:m + 1], mx8[:, 0:1])
                        nc.vector.tensor_copy(i0[:, m:m + 1], ix8[:, 0:1])
                    else:
                        gB = smallp.tile([P, 1], dt.float32, tag="gB", name="gB")
                        iBf = smallp.tile([P, 1], dt.float32, tag="iBf", name="iBf")
                        mask = smallp.tile([P, 1], dt.uint8, tag="mask", name="mask")
                        gfin = smallp.tile([P, 1], dt.float32, tag="gfin", name="gfin")
                        ifin = smallp.tile([P, 1], dt.float32, tag="ifin", name="ifin")
                        d2t_ = smallp.tile([P, 1], dt.float32, tag="d2t_", name="d2t_")
                        dout = opool.tile([P, 1], dt.float32, tag="dout", name="dout")
                        iout = opool.tile([P, 1], dt.int32, tag="iout", name="iout")
                        nc.vector.tensor_copy(gB[:], mx8[:, 0:1])
                        nc.vector.tensor_copy(iBf[:], ix8[:, 0:1])
                        # global index for half-1 hits
                        nc.vector.tensor_scalar_add(iBf[:], iBf[:], float(HW))
                        nc.vector.tensor_tensor(
                            mask[:], g0[:, m:m + 1], gB[:], Alu.is_ge
                        )
                        nc.vector.tensor_tensor(
                            gfin[:], g0[:, m:m + 1], gB[:], Alu.max
                        )
                        nc.vector.select(ifin[:], mask[:], i0[:, m:m + 1], iBf[:])
                        # d2 = relu(a2 - g);  dist = sqrt(d2)
                        nc.scalar.activation(
                            d2t_[:], gfin[:], Act.Relu,
                            bias=a2sb[:, m:m + 1], scale=-1.0,
                        )
                        nc.scalar.activation(dout[:], d2t_[:], Act.Sqrt)
                        nc.vector.tensor_copy(iout[:], ifin[:])
                        nc.sync.dma_start(out=out_d[ms:ms + P, :], in_=dout[:])
                        nc.sync.dma_start(out=out_i[ms:ms + P, :], in_=iout[:])
    return _split_excess_waits(nc)


def _split_excess_waits(nc, max_waits=1):
    """walrus in this container caps sync waits per instruction; move excess
    waits onto preceding same-engine no-ops (waits are sem>=value, so
    splitting across program-ordered instructions is equivalent)."""
    import concourse.mybir as mybir

    uid = [0]
    for f in nc.m.functions:
        for bb in f.blocks:
            insts = list(bb.instructions)
            out = []
            changed = False
            for inst in insts:
                si = inst.sync_info
                ws = list(si.on_wait) if si is not None and si.on_wait else []
                if len(ws) > max_waits:
                    changed = True
                    carry, keep = ws[:-max_waits], ws[-max_waits:]
                    for j in range(0, len(carry), max_waits):
                        nop = mybir.InstNoOp(
                            name=f"waitnop_{uid[0]}", ins=[], outs=[]
                        )
                        uid[0] += 1
                        nop.engine = inst.engine
                        nop.sync_info = mybir.SyncInfo(
                            on_wait=carry[j:j + max_waits], on_update=[]
                        )
                        out.append(nop)
                    inst.sync_info = mybir.SyncInfo(
                        on_wait=keep, on_update=list(si.on_update or [])
                    )
                out.append(inst)
            if changed:
                bb.instructions = out
    return nc


def _host_prep(desc1, desc2, B1c):
    d1 = np.ascontiguousarray(desc1, dtype=np.float32) * 2.0
    d2 = np.ascontiguousarray(desc2, dtype=np.float32)

    def split(x):
        h = x.astype(np.float16)
        l = (x - h.astype(np.float32)).astype(np.float16)
        return h, l

    d1h, d1l = split(d1)
    d2h, d2l = split(d2)
    d2t_h = np.ascontiguousarray(d2h.T)
    d2t_l = np.ascontiguousarray(d2l.T)
    b2 = (d2.astype(np.float64) ** 2).sum(1).astype(np.float32)
    b2h_ = b2.astype(np.float16)
    b2l_ = (b2 - b2h_.astype(np.float32)).astype(np.float16)
    b2a = np.ascontiguousarray(np.stack([b2h_, b2l_], axis=0))
    a2 = ((desc1.astype(np.float64)) ** 2).sum(1).astype(np.float32)

    in_maps = []
    ncores = d1.shape[0] // B1c
    for c in range(ncores):
        rows = slice(c * B1c, (c + 1) * B1c)
        in_maps.append({
            "d1t_h": np.ascontiguousarray(d1h[rows].T),
            "d1t_l": np.ascontiguousarray(d1l[rows].T),
            "d2t_h": d2t_h,
            "d2t_l": d2t_l,
            "b2a": b2a,
            "a2": np.ascontiguousarray(a2[rows, None]),
        })
    return in_maps


_NC_CACHE = {}


def _get_nc():
    key = "full"
    if key not in _NC_CACHE:
        _NC_CACHE[key] = build_nc()
    return _NC_CACHE[key]


def run_on_cores(desc1, desc2, trace=False):
    from concourse.bass_utils import run_bass_kernel_spmd

    B1c = desc1.shape[0] // N_CORES
    nc = _get_nc()
    in_maps = _host_prep(desc1, desc2, B1c)
    res = run_bass_kernel_spmd(
        nc, in_maps, list(range(N_CORES)), trace=trace,
    )
    dists = np.concatenate([r["out_d"] for r in res.results], axis=0)
    idxs = np.concatenate([r["out_i"][:, 0] for r in res.results], axis=0)
    return dists, idxs, res


def kernel(desc1, desc2):
    desc1 = np.asarray(desc1)
    desc2 = np.asarray(desc2)
    dists, idxs, _ = run_on_cores(desc1, desc2)
    n = idxs.shape[0]
    matches = np.stack(
        [np.arange(n, dtype=np.int32), idxs.astype(np.int32)], axis=1
    )
    return dists.astype(np.float32), matches


# revision 9
# speedup vs baseline: 1.3823x; 1.3823x over previous
"""Trainium2 Bass kernel for DescriptorMatcher (nn): row-sharded cdist + row min/argmin.

Strategy (8 NeuronCores, desc1 rows sharded 2048/core, desc2 replicated):
  - Distance identity: argmin_j d2(i,j) = argmax_j s(i,j),  s = 2*a.b_j - ||b_j||^2,
    d2 = ||a_i||^2 - max_j s.
  - s is produced tile-by-tile in PSUM by the TensorEngine: desc1 is pre-scaled
    by 2 and split into fp16 hi/lo halves (three matmul terms hi*hi, hi*lo,
    lo*hi -> ~1e-5 absolute accuracy at full bf16-rate PE throughput), plus a
    K=2 augmentation matmul that subtracts ||b||^2 (fp16 hi/lo rows times -1).
  - ScalarEngine evacuates PSUM->SBUF; VectorEngine does per-tile row-max
    (tensor_reduce), then max8 + max_index recover the row argmax index.
  - Outputs per core: match_dists slice [2048,1] fp32, argmin slice [2048,1] i32.
"""

import sys

import numpy as np

if "/opt/trn_rl_repo" not in sys.path:
    sys.path.append("/opt/trn_rl_repo")

B1, B2, D = 16384, 16384, 256
N_CORES = 8
P = 128          # partitions
NT = 512         # psum tile free dim (one bank)
N_HALF = 2       # process desc2 columns in halves to fit SBUF


def build_nc(B1c=B1 // N_CORES, B2_=B2, D_=D, repeat=1):
    import concourse.bass as bass
    import concourse.mybir as mybir
    import concourse.tile as tile

    dt = mybir.dt
    Alu = mybir.AluOpType
    Act = mybir.ActivationFunctionType
    AxX = mybir.AxisListType.X

    KC = D_ // P                 # contraction chunks (2)
    HW = B2_ // N_HALF           # half width (8192)
    NTH = HW // NT               # psum tiles per half (16)
    MB = B1c // P                # row blocks per core (16)
    GRP = min(4, NTH)            # psum tiles per matmul group
    assert NTH % GRP == 0 and NTH >= 8

    nc = bass.Bass("TRN2", target_bir_lowering=False, debug=False)

    d1t_h = nc.dram_tensor("d1t_h", [D_, B1c], dt.float16, kind="ExternalInput").ap()
    d1t_l = nc.dram_tensor("d1t_l", [D_, B1c], dt.float16, kind="ExternalInput").ap()
    d2t_h = nc.dram_tensor("d2t_h", [D_, B2_], dt.float16, kind="ExternalInput").ap()
    d2t_l = nc.dram_tensor("d2t_l", [D_, B2_], dt.float16, kind="ExternalInput").ap()
    b2a = nc.dram_tensor("b2a", [2, B2_], dt.float16, kind="ExternalInput").ap()
    a2 = nc.dram_tensor("a2", [B1c, 1], dt.float32, kind="ExternalInput").ap()
    out_d = nc.dram_tensor("out_d", [B1c, 1], dt.float32, kind="ExternalOutput").ap()
    out_i = nc.dram_tensor("out_i", [B1c, 1], dt.int32, kind="ExternalOutput").ap()

    with tile.TileContext(nc) as tc:
        with (
            tc.tile_pool(name="const", bufs=1) as cpool,
            tc.tile_pool(name="rhs", bufs=1) as rpool,
            tc.tile_pool(name="sprime", bufs=1) as spool,
            tc.tile_pool(name="mmax", bufs=2) as mpool,
            tc.tile_pool(name="small", bufs=8) as smallp,
            tc.tile_pool(name="outs", bufs=4) as opool,
            tc.tile_pool(name="psum", bufs=8, space="PSUM") as ppool,
        ):
            # --- persistent SBUF: lhsT (fp16 hi/lo per K-chunk), a2, aug consts
            lhs = {}
            for part, src in (("h", d1t_h), ("l", d1t_l)):
                for k in range(KC):
                    t = cpool.tile([P, B1c], dt.float16, tag=f"lhs_{part}{k}",
                                   name=f"lhs_{part}{k}")
                    nc.sync.dma_start(out=t[:], in_=src[k * P:(k + 1) * P, :])
                    lhs[(part, k)] = t
            a2sb = cpool.tile([P, MB], dt.float32, tag="a2sb", name="a2sb")
            nc.sync.dma_start(
                out=a2sb[:], in_=a2.rearrange("(m p) o -> p (m o)", p=P)
            )
            negones = cpool.tile([P, P], dt.float16, tag="negones", name="negones")
            nc.vector.memset(negones[:], -1.0)
            g0 = cpool.tile([P, MB], dt.float32, tag="g0", name="g0")
            i0 = cpool.tile([P, MB], dt.float32, tag="i0", name="i0")

            for rep_h in range(repeat * N_HALF):
                h = rep_h % N_HALF
                cs = h * HW  # column start
                rhs = {}
                for part, src in (("h", d2t_h), ("l", d2t_l)):
                    for k in range(KC):
                        t = rpool.tile([P, HW], dt.float16, tag=f"rhs_{part}{k}",
                                       name=f"rhs_{part}{k}")
                        nc.sync.dma_start(
                            out=t[:], in_=src[k * P:(k + 1) * P, cs:cs + HW]
                        )
                        rhs[(part, k)] = t
                aug = rpool.tile([P, HW], dt.float16, tag="aug", name="aug")
                nc.sync.dma_start(out=aug[0:2, :], in_=b2a[:, cs:cs + HW])

                for m in range(MB):
                    ms = m * P
                    sp = spool.tile([P, HW], dt.float32, tag="sp", name="sp")
                    mt = mpool.tile([P, NTH], dt.float32, tag="mt", name="mt")
                    psums = [None] * NTH
                    for g in range(NTH // GRP):
                        tiles = list(range(g * GRP, (g + 1) * GRP))
                        for n in tiles:
                            psums[n] = ppool.tile(
                                [P, NT], dt.float32, tag="ps", name="ps"
                            )
                        # aug matmul: psum = -(b2h + b2l) per column
                        for n in tiles:
                            nc.tensor.matmul(
                                psums[n][:],
                                negones[0:2, :],
                                aug[0:2, n * NT:(n + 1) * NT],
                                start=True, stop=False,
                            )
                        # weight-major data matmuls: lhsT loaded once per group
                        steps = [
                            ("h", 0, ("h", "l")),
                            ("h", 1, ("h", "l")),
                            ("l", 0, ("h",)),
                            ("l", 1, ("h",)),
                        ]
                        for si, (wp, wk, rparts) in enumerate(steps):
                            last_step = si == len(steps) - 1
                            w = lhs[(wp, wk)][:, ms:ms + P]
                            for ri, rp in enumerate(rparts):
                                last = last_step and ri == len(rparts) - 1
                                for n in tiles:
                                    r = rhs[(rp, wk)][:, n * NT:(n + 1) * NT]
                                    nc.tensor.matmul(
                                        psums[n][:], w, r,
                                        start=False, stop=last,
                                    )
                        for n in tiles:
                            nc.scalar.copy(sp[:, n * NT:(n + 1) * NT], psums[n][:])
                            nc.vector.tensor_reduce(
                                mt[:, n:n + 1], psums[n][:], AxX, Alu.max
                            )
                    mx8 = smallp.tile([P, 8], dt.float32, tag="mx8", name="mx8")
                    ix8 = smallp.tile([P, 8], dt.uint32, tag="ix8", name="ix8")
                    nc.vector.max(mx8[:], mt[:])
                    nc.vector.max_index(ix8[:], mx8[:], sp[:])
                    if h == 0:
                        nc.vector.tensor_copy(g0[:, m:m + 1], mx8[:, 0:1])
                        nc.vector.tensor_copy(i0[:, m:m + 1], ix8[:, 0:1])
                    else:
                        gB = smallp.tile([P, 1], dt.float32, tag="gB", name="gB")
                        iBf = smallp.tile([P, 1], dt.float32, tag="iBf", name="iBf")
                        mask = smallp.tile([P, 1], dt.uint8, tag="mask", name="mask")
                        gfin = smallp.tile([P, 1], dt.float32, tag="gfin", name="gfin")
                        ifin = smallp.tile([P, 1], dt.float32, tag="ifin", name="ifin")
                        d2t_ = smallp.tile([P, 1], dt.float32, tag="d2t_", name="d2t_")
                        dout = opool.tile([P, 1], dt.float32, tag="dout", name="dout")
                        iout = opool.tile([P, 1], dt.int32, tag="iout", name="iout")
                        nc.vector.tensor_copy(gB[:], mx8[:, 0:1])
                        nc.vector.tensor_copy(iBf[:], ix8[:, 0:1])
                        # global index for half-1 hits
                        nc.vector.tensor_scalar_add(iBf[:], iBf[:], float(HW))
                        nc.vector.tensor_tensor(
                            mask[:], g0[:, m:m + 1], gB[:], Alu.is_ge
                        )
                        nc.vector.tensor_tensor(
                            gfin[:], g0[:, m:m + 1], gB[:], Alu.max
                        )
                        nc.vector.select(ifin[:], mask[:], i0[:, m:m + 1], iBf[:])
                        # d2 = relu(a2 - g);  dist = sqrt(d2)
                        nc.scalar.activation(
                            d2t_[:], gfin[:], Act.Relu,
                            bias=a2sb[:, m:m + 1], scale=-1.0,
                        )
                        nc.scalar.activation(dout[:], d2t_[:], Act.Sqrt)
                        nc.vector.tensor_copy(iout[:], ifin[:])
                        nc.sync.dma_start(out=out_d[ms:ms + P, :], in_=dout[:])
                        nc.sync.dma_start(out=out_i[ms:ms + P, :], in_=iout[:])
    return _split_excess_waits(nc)


def _split_excess_waits(nc, max_waits=1):
    """walrus in this container caps sync waits per instruction; move excess
    waits onto preceding same-engine no-ops (waits are sem>=value, so
    splitting across program-ordered instructions is equivalent)."""
    import concourse.mybir as mybir

    uid = [0]
    for f in nc.m.functions:
        for bb in f.blocks:
            insts = list(bb.instructions)
            out = []
            changed = False
            for inst in insts:
                si = inst.sync_info
                ws = list(si.on_wait) if si is not None and si.on_wait else []
                if len(ws) > max_waits:
                    changed = True
                    carry, keep = ws[:-max_waits], ws[-max_waits:]
                    for j in range(0, len(carry), max_waits):
                        nop = mybir.InstNoOp(
                            name=f"waitnop_{uid[0]}", ins=[], outs=[]
                        )
                        uid[0] += 1
                        nop.engine = inst.engine
                        nop.sync_info = mybir.SyncInfo(
                            on_wait=carry[j:j + max_waits], on_update=[]
                        )
                        out.append(nop)
                    inst.sync_info = mybir.SyncInfo(
                        on_wait=keep, on_update=list(si.on_update or [])
                    )
                out.append(inst)
            if changed:
                bb.instructions = out
    return nc


def _host_prep(desc1, desc2, B1c):
    d1 = np.ascontiguousarray(desc1, dtype=np.float32) * 2.0
    d2 = np.ascontiguousarray(desc2, dtype=np.float32)

    def split(x):
        h = x.astype(np.float16)
        l = (x - h.astype(np.float32)).astype(np.float16)
        return h, l

    d1h, d1l = split(d1)
    d2h, d2l = split(d2)
    d2t_h = np.ascontiguousarray(d2h.T)
    d2t_l = np.ascontiguousarray(d2l.T)
    b2 = (d2.astype(np.float64) ** 2).sum(1).astype(np.float32)
    b2h_ = b2.astype(np.float16)
    b2l_ = (b2 - b2h_.astype(np.float32)).astype(np.float16)
    b2a = np.ascontiguousarray(np.stack([b2h_, b2l_], axis=0))
    a2 = ((desc1.astype(np.float64)) ** 2).sum(1).astype(np.float32)

    in_maps = []
    ncores = d1.shape[0] // B1c
    for c in range(ncores):
        rows = slice(c * B1c, (c + 1) * B1c)
        in_maps.append({
            "d1t_h": np.ascontiguousarray(d1h[rows].T),
            "d1t_l": np.ascontiguousarray(d1l[rows].T),
            "d2t_h": d2t_h,
            "d2t_l": d2t_l,
            "b2a": b2a,
            "a2": np.ascontiguousarray(a2[rows, None]),
        })
    return in_maps


_NC_CACHE = {}


def _get_nc():
    key = "full"
    if key not in _NC_CACHE:
        _NC_CACHE[key] = build_nc()
    return _NC_CACHE[key]


def run_on_cores(desc1, desc2, trace=False):
    from concourse.bass_utils import run_bass_kernel_spmd

    B1c = desc1.shape[0] // N_CORES
    nc = _get_nc()
    in_maps = _host_prep(desc1, desc2, B1c)
    res = run_bass_kernel_spmd(
        nc, in_maps, list(range(N_CORES)), trace=trace,
    )
    dists = np.concatenate([r["out_d"] for r in res.results], axis=0)
    idxs = np.concatenate([r["out_i"][:, 0] for r in res.results], axis=0)
    return dists, idxs, res


def kernel(desc1, desc2):
    desc1 = np.asarray(desc1)
    desc2 = np.asarray(desc2)
    dists, idxs, _ = run_on_cores(desc1, desc2)
    n = idxs.shape[0]
    matches = np.stack(
        [np.arange(n, dtype=np.int32), idxs.astype(np.int32)], axis=1
    )
    return dists.astype(np.float32), matches
